# revision 21
# baseline (speedup 1.0000x reference)
"""BiasedMultiHeadAttention Trainium2 kernel.

Sharding: 8 cores = (batch b, query-half qh). Each core computes the full
pipeline for its 512 query rows of batch b (K/V projections for the batch
are duplicated across the 2 cores sharing it). No collectives.

Device layout trick: per-core x rows are host-rolled so the core's query
block is always rows 0..511 -> one SPMD program for all 8 cores; bias/mask
are rolled consistently (softmax sum order irrelevant).

Math folding (host, exact):
  xn_aff = ln(x)*g + b folded into weights:  w_eff[i,o] = w[o,i]*ln_g[i]
  b_eff[o] = (w @ ln_b + b)[o];  Q additionally scaled by SCALE/gate_h.

Perf structure (v4):
  - Q/K/V projections in fp8e4 DoubleRow (256-deep contraction = 2x MACs);
    weights host-prescaled by 2^a into e4m3 range, compensated in the
    PSUM->SBUF copy scale.
  - Projection emission is INTERLEAVED with attention head-pairs: after
    K/Q chunks 0-1 and V(half 0) the t=0 attention starts, and each
    remaining K/Q chunk is emitted between head-pairs. The ACT exp train
    (the per-core softmax floor) starts ~40us earlier than with separate
    phases.
  - The bias tile is preloaded into PSUM by an identity matmul
    (start=True), QK accumulates on top, EXP reads PSUM and writes fp8
    at tiles (shifted by AT_SHIFT) that pair two key chunks for
    DoubleRow AV.
  - PSUM plan (8 banks): transposes 1 + proj ring 2 + per-head score
    halves 3 + av pair 2.
"""

import numpy as np
import ml_dtypes

import concourse.bass as bass
import concourse.tile as tile
import concourse.mybir as mybir
from concourse import bacc
from concourse.bass_utils import run_bass_kernel_spmd

B, L, E, H = 4, 1024, 1024, 16
D = E // H
SCALE = D**-0.5
EPS = 1e-5
NCORES = 8
QL = 512  # query rows per core
PT = 128  # partitions
NL = L // PT  # 8 l-chunks
NE = E // PT  # 8 e-chunks
HP = H // 2  # 8 head pairs
NM = NL // 2  # 4 key-chunk pairs

F32 = mybir.dt.float32
BF16 = mybir.dt.bfloat16
F8 = mybir.dt.float8e4
I32 = mybir.dt.int32
BF_NP = ml_dtypes.bfloat16
F8_NP = ml_dtypes.float8_e4m3fn
DR = mybir.MatmulPerfMode.DoubleRow

# host prescale exponents for fp8 weights (compensated in psum->sbuf copy)
QW_S = 32.0   # wq (incl. SCALE/gate fold): sigma ~0.008 -> 0.25
KW_S = 8.0    # wk: sigma ~0.031 -> 0.25
VW_S = 8.0
AT_SHIFT = -3.0   # ACT bias on the exp; keeps fp8 at below e4m3 max

LAST_RESULT = None  # BassKernelResults of the most recent run (for test.py)


def _f8(a):
    return np.clip(np.asarray(a, np.float32), -240.0, 240.0).astype(F8_NP)


def _build_nc(gates, use_pbias, use_mask, dump=()):
    """Build the single-core Bass program (same NEFF for all 8 cores)."""
    nc = bacc.Bacc("TRN2", target_bir_lowering=False, debug=False)
    dump_d = {}

    def dump_tile(name, ap):
        if name in dump:
            d = nc.dram_tensor("d_" + name, list(ap.shape), ap.dtype,
                               kind="ExternalOutput")
            nc.sync.dma_start(d[tuple(slice(None) for _ in ap.shape)], ap)
            dump_d[name] = d

    x_d = nc.dram_tensor("xc", [PT, NL, L], F32, kind="ExternalInput")
    bias_d = nc.dram_tensor("biasc", [H, L, QL], BF16, kind="ExternalInput")
    id_d = nc.dram_tensor("idm", [PT, PT], BF16, kind="ExternalInput")
    wq_d = nc.dram_tensor("wqt", [PT, NE, E], F8, kind="ExternalInput")
    wk_d = nc.dram_tensor("wkt", [PT, NE, E], F8, kind="ExternalInput")
    wv_d = nc.dram_tensor("wvt", [PT, NE, E], F8, kind="ExternalInput")
    wo_d = nc.dram_tensor("wot", [PT, NE, E], BF16, kind="ExternalInput")
    pb_d = {}
    for name, use in zip("qkvo", use_pbias):
        if use:
            pb_d[name] = nc.dram_tensor(f"b{name}e", [1, E], BF16,
                                        kind="ExternalInput")
    if use_mask:
        km_d = nc.dram_tensor("kmc", [PT, NL], F32, kind="ExternalInput")
        mq_d = nc.dram_tensor("mqc", [1, QL], F32, kind="ExternalInput")
    y_d = nc.dram_tensor("yc", [QL, E], F32, kind="ExternalOutput")

    with tile.TileContext(nc) as tc:
        with (
            tc.tile_pool(name="persist", bufs=1) as pp,
            tc.tile_pool(name="consts", bufs=1) as cp,
        ):
            # ---- constants ----
            ident = cp.tile([PT, PT], BF16)
            nc.sync.dma_start(ident, id_d[:, :])
            ones_row = cp.tile([1, L], BF16)
            nc.vector.memset(ones_row, 1.0)
            eps_t = cp.tile([PT, 1], F32)
            nc.vector.memset(eps_t, EPS)
            shift_t = cp.tile([PT, 1], F32)
            nc.vector.memset(shift_t, AT_SHIFT)
            if use_mask:
                km_sb = cp.tile([PT, NL], F32)
                nc.sync.dma_start(km_sb, km_d[:, :])
                mqb = cp.tile([64, QL], F32)
                nc.gpsimd.dma_start(mqb,
                                    mq_d[0:1, :].partition_broadcast(64))

            # ---- resident tensors ----
            # x split by column halves: bn_stats consumes halves, so the
            # LayerNorm chain starts after 0.25MB instead of 0.5MB.
            x_sb = pp.tile([PT, NL, L], F32)
            for lt in range(NL):
                for ch in range(2):
                    nc.sync.dma_start(x_sb[:, lt, ch * 512:(ch + 1) * 512],
                                      x_d[:, lt, ch * 512:(ch + 1) * 512])
            # weights early so DMA overlaps phase 1
            wq_sb = pp.tile([PT, NE, E], F8)
            nc.sync.dma_start(wq_sb, wq_d[:, :, :])
            wk_sb = pp.tile([PT, NE, E], F8)
            nc.sync.dma_start(wk_sb, wk_d[:, :, :])
            wv_sb = pp.tile([PT, NE, E], F8)
            nc.sync.dma_start(wv_sb, wv_d[:, :, :])
            wo_sb = pp.tile([PT, NE, E], BF16)
            nc.sync.dma_start(wo_sb, wo_d[:, :, :])
            # K^T zero-padded per head parity: full-K=128 QK matmuls with
            # the other head's rows zeroed.
            kTzA = pp.tile([PT, NE, L], BF16)
            kTzB = pp.tile([PT, NE, L], BF16)
            nc.gpsimd.memset(kTzA[64:128, :, :], 0.0)
            nc.gpsimd.memset(kTzB[0:64, :, :], 0.0)
            # V | ones col per head, paired key chunks for DoubleRow AV:
            # v3[:, m, s, h, :] = V rows of key chunk 2m+s
            v3 = pp.tile([PT, NM, 2, H, 65], F8)
            qT = pp.tile([PT, NE, QL], BF16)    # Q^T (scaled) [e_q, q]
            oT = pp.tile([PT, NE, QL], BF16)    # attnout^T (normalized)
            nc.vector.memset(v3[:, :, :, :, 64:65], 1.0)  # ones cols
            pbr = {}
            for name in pb_d:
                pbr[name] = cp.tile([1, E], BF16)
                nc.sync.dma_start(pbr[name], pb_d[name][:, :])

            # ========= Phases 1-3 fused: LN+transpose, projections =======
            # interleaved with attention head-pairs.
            with (
                tc.tile_pool(name="ln", bufs=3) as lp,
                tc.tile_pool(name="xnt", bufs=1) as xp,
                tc.tile_pool(name="pst", bufs=1, space="PSUM") as ptp,
                tc.tile_pool(name="prj", bufs=2, space="PSUM") as prp,
                tc.tile_pool(name="sc", bufs=3, space="PSUM") as scp,
                tc.tile_pool(name="av", bufs=2, space="PSUM") as avp,
                tc.tile_pool(name="bias", bufs=8) as bp,
                tc.tile_pool(name="att2", bufs=6) as a2p,
                tc.tile_pool(name="oo", bufs=6) as oop,
                tc.tile_pool(name="recd", bufs=4, space="DRAM") as rdp,
            ):
                xnT = xp.tile([PT, NE, L], F8)  # xn^T [e, l] fp8

                def emit_ln(lt):
                    xr = x_sb[:, lt, :].rearrange("p (s d) -> p s d", s=2)
                    stats = lp.tile([PT, 2, 6], F32, tag="stats")
                    for sg in range(2):
                        nc.vector.bn_stats(stats[:, sg, :], xr[:, sg, :])
                    mv = lp.tile([PT, 2], F32, tag="mv")
                    nc.vector.bn_aggr(mv, stats)
                    sd = lp.tile([PT, 1], F32, tag="sd")
                    nc.scalar.activation(sd, mv[:, 1:2],
                                         mybir.ActivationFunctionType.Sqrt,
                                         bias=eps_t)
                    rs = lp.tile([PT, 1], F32, tag="rs")
                    nc.vector.reciprocal(rs, sd)
                    xnb = lp.tile([PT, L], BF16, tag="xnb")
                    nc.vector.tensor_scalar(
                        out=xnb, in0=x_sb[:, lt, :], scalar1=mv[:, 0:1],
                        scalar2=rs, op0=mybir.AluOpType.subtract,
                        op1=mybir.AluOpType.mult)
                    psT = ptp.tile([PT, L], BF16, tag="psT")
                    for j in range(NE):
                        nc.tensor.transpose(
                            psT[:, j * PT:(j + 1) * PT],
                            xnb[:, j * PT:(j + 1) * PT], ident)
                    nc.scalar.copy(
                        xnT[:, :, lt * PT:(lt + 1) * PT],
                        psT.rearrange("p (j l) -> p j l", j=NE))

                def emit_kq(ot):
                    osl = slice(ot * PT, (ot + 1) * PT)
                    for nh in range(2):  # l halves
                        nsl = slice(nh * 512, (nh + 1) * 512)
                        ps = prp.tile([PT, 512], F32, tag="pp")
                        for j in range(4):
                            nc.tensor.matmul(
                                ps, wk_sb[:, 2 * j:2 * j + 2, osl],
                                xnT[:, 2 * j:2 * j + 2, nsl],
                                start=(j == 0),
                                stop=(j == 3 and "k" not in pbr),
                                perf_mode=DR)
                        if "k" in pbr:
                            nc.tensor.matmul(ps, pbr["k"][:, osl],
                                             ones_row[:, 0:512],
                                             start=False, stop=True)
                        nc.scalar.mul(kTzA[0:64, ot, nsl], ps[0:64, :],
                                      1.0 / KW_S)
                        nc.vector.tensor_scalar_mul(
                            kTzB[64:128, ot, nsl], ps[64:128, :],
                            1.0 / KW_S)
                    # Q (first 512 rolled rows only)
                    psq = prp.tile([PT, 512], F32, tag="pp")
                    for j in range(4):
                        nc.tensor.matmul(
                            psq, wq_sb[:, 2 * j:2 * j + 2, osl],
                            xnT[:, 2 * j:2 * j + 2, 0:512],
                            start=(j == 0),
                            stop=(j == 3 and "q" not in pbr),
                            perf_mode=DR)
                    if "q" in pbr:
                        nc.tensor.matmul(psq, pbr["q"][:, osl],
                                         ones_row[:, 0:512],
                                         start=False, stop=True)
                    nc.scalar.mul(qT[:, ot, 0:512], psq, 1.0 / QW_S)

                def emit_v(lt, vh):
                    lsl = slice(lt * PT, (lt + 1) * PT)
                    vsl = slice(vh * 512, (vh + 1) * 512)
                    psv = prp.tile([PT, 512], F32, tag="pp")
                    for j in range(4):
                        nc.tensor.matmul(
                            psv, xnT[:, 2 * j:2 * j + 2, lsl],
                            wv_sb[:, 2 * j:2 * j + 2, vsl],
                            start=(j == 0),
                            stop=(j == 3 and "v" not in pbr),
                            perf_mode=DR)
                    if "v" in pbr:
                        nc.tensor.matmul(psv, ones_row[:, 0:PT],
                                         pbr["v"][:, vsl],
                                         start=False, stop=True)
                    nc.vector.tensor_scalar_mul(
                        v3[:, lt // 2, lt % 2, vh * 8:(vh + 1) * 8, 0:64],
                        psv.rearrange("p (h d) -> p h d", h=8),
                        1.0 / VW_S)

                def emit_att(t):
                    hA, hB = 2 * t, 2 * t + 1
                    avA = avp.tile([65, QL], F32, tag="av")
                    avB = avp.tile([65, QL], F32, tag="av")
                    at2 = None
                    for c in range(NL):
                        csl = slice(c * PT, (c + 1) * PT)
                        bt = bp.tile([PT, 2, QL], BF16, tag="bt")
                        nc.sync.dma_start(
                            bt, bias_d[hA:hB + 1, csl, :].rearrange(
                                "h p q -> p h q"))
                        if c % 2 == 0:
                            at2 = a2p.tile([PT, 2, 2, QL], F8, tag="at")
                        kmb = (km_sb[:, c:c + 1] if use_mask
                               else shift_t[:, 0:1])
                        for hi, h, ktz in ((0, hA, kTzA), (1, hB, kTzB)):
                            psh = scp.tile([PT, QL], F32, tag="ps")
                            nc.tensor.matmul(psh, ident, bt[:, hi, :],
                                             start=True, stop=False)
                            nc.tensor.matmul(psh, ktz[:, t, csl],
                                             qT[:, t, :],
                                             start=False, stop=True)
                            nc.scalar.activation(
                                at2[:, hi, c % 2, :], psh,
                                mybir.ActivationFunctionType.Exp,
                                bias=kmb, scale=gates[h])
                        if c % 2 == 1:
                            m = c // 2
                            nc.tensor.matmul(
                                avA, v3[:, m, :, hA, :], at2[:, 0, :, :],
                                start=(m == 0), stop=(m == NM - 1),
                                perf_mode=DR)
                            nc.tensor.matmul(
                                avB, v3[:, m, :, hB, :], at2[:, 1, :, :],
                                start=(m == 0), stop=(m == NM - 1),
                                perf_mode=DR)
                    # normalize: SBUF bounce frees the PSUM bank fast;
                    # rowsum -> recip -> broadcast -> mul
                    for hi, av in enumerate((avA, avB)):
                        avs = oop.tile([65, QL], F32, tag="avs")
                        nc.vector.tensor_copy(avs, av)
                        recd = rdp.tile([1, QL], F32, tag="recd")
                        nc.sync.dma_start(recd, avs[64:65, :])
                        rbs = oop.tile([64, QL], F32, tag="rbs")
                        nc.gpsimd.dma_start(
                            rbs, recd[0:1, :].partition_broadcast(64))
                        nc.vector.reciprocal_approx_fast(out=rbs, in_=rbs)
                        if use_mask:
                            nc.vector.tensor_mul(rbs, rbs, mqb)
                        if hi == 0:
                            nc.vector.tensor_mul(oT[0:64, t, :],
                                                 avs[0:64, :], rbs)
                        else:
                            ot_odd = oop.tile([64, QL], BF16, tag="oo")
                            nc.vector.tensor_mul(ot_odd, avs[0:64, :], rbs)
                            nc.sync.dma_start(oT[64:128, t, :], ot_odd)

                # ---- emission order: attention t=0 starts right after
                # K/Q chunks 0-1 and the V half it needs ----
                for lt in range(NL):
                    emit_ln(lt)
                emit_kq(0)
                emit_kq(1)
                for lt in range(NL):
                    emit_v(lt, 0)
                emit_att(0)
                emit_kq(2)
                for lt in range(NL):
                    emit_v(lt, 1)
                emit_att(1)
                for t in range(2, HP):
                    if t + 1 < HP:
                        emit_kq(t + 1)
                    emit_att(t)

            dump_tile("oT", oT[:, :, :])

            # ====== Phase 4: out-proj directly in [q, e] + residual ======
            with (
                tc.tile_pool(name="fp", bufs=2, space="PSUM") as fpp,
                tc.tile_pool(name="yo", bufs=2) as yop,
            ):
                for qb in range(4):
                    qsl = slice(qb * PT, (qb + 1) * PT)
                    psf = fpp.tile([PT, E], F32, tag="pf")
                    for eh in range(2):
                        esl = slice(eh * 512, (eh + 1) * 512)
                        for ic in range(NE):
                            nc.tensor.matmul(
                                psf[:, esl], oT[:, ic, qsl],
                                wo_sb[:, ic, esl], start=(ic == 0),
                                stop=(ic == NE - 1 and "o" not in pbr))
                        if "o" in pbr:
                            nc.tensor.matmul(psf[:, esl],
                                             ones_row[0:1, 0:PT],
                                             pbr["o"][:, esl],
                                             start=False, stop=True)
                    y_sb = yop.tile([PT, E], F32, tag="y")
                    nc.vector.tensor_add(y_sb, psf, x_sb[:, qb, :])
                    nc.sync.dma_start(y_d[qsl, :], y_sb)
    return nc


def _prep_inputs(x, bias, mask, wq, bq, wk, bk, wv, bv, wo, bo, gate,
                 ln_g, ln_b):
    """Host-side folding + per-core sharding. Returns (in_maps, meta)."""
    gate = np.asarray(gate, np.float32)
    ln_g = np.asarray(ln_g, np.float32)
    ln_b = np.asarray(ln_b, np.float32)
    grep = np.repeat(gate, D)  # [E]
    safe_gate = bool(np.all(np.abs(gate) > 1e-6))
    if safe_gate:
        qscale = (SCALE / grep).astype(np.float32)
        exp_scales = [float(g) for g in gate]
    else:
        # fold gate into bias on host instead (gate ~ 0 edge case)
        qscale = np.full(E, SCALE, np.float32)
        exp_scales = [1.0] * H

    wqt = _f8(np.asarray(wq).T * ln_g[:, None] * qscale[None, :] * QW_S)
    wkt = _f8(np.asarray(wk).T * ln_g[:, None] * KW_S)
    wvt = _f8(np.asarray(wv).T * ln_g[:, None] * VW_S)
    wot = np.asarray(wo).T.astype(BF_NP)
    bqe = ((np.asarray(wq) @ ln_b + np.asarray(bq)) * qscale * QW_S
           ).astype(np.float32)
    bke = ((np.asarray(wk) @ ln_b + np.asarray(bk)) * KW_S).astype(np.float32)
    bve = ((np.asarray(wv) @ ln_b + np.asarray(bv)) * VW_S).astype(np.float32)
    boe = np.asarray(bo, np.float32)
    use_pbias = tuple(bool(np.any(b)) for b in (bqe, bke, bve, boe))

    mask = np.asarray(mask, np.int32)
    use_mask = not bool(np.all(mask == 1))

    def wfmt(w):  # [E_in, E_out] -> [128, 8, E]
        return np.ascontiguousarray(
            w.reshape(NE, PT, E).transpose(1, 0, 2))

    shared = {"wqt": wfmt(wqt), "wkt": wfmt(wkt), "wvt": wfmt(wvt),
              "wot": wfmt(wot),
              "idm": np.eye(PT, dtype=BF_NP)}
    for name, use, b in zip("qkvo", use_pbias, (bqe, bke, bve, boe)):
        if use:
            shared[f"b{name}e"] = b.reshape(1, E).astype(BF_NP)

    x = np.asarray(x, np.float32)
    bias = np.asarray(bias, np.float32)
    in_maps = []
    for c in range(NCORES):
        b_idx, qh = divmod(c, 2)
        q0 = qh * QL
        xr = np.roll(x[b_idx], -q0, axis=0)  # query block first
        m = {}
        m.update(shared)
        m["xc"] = np.ascontiguousarray(
            xr.reshape(NL, PT, L).transpose(1, 0, 2))
        bs = bias[b_idx][:, q0:q0 + QL, :]  # [H, QL, L]
        bs = np.roll(bs, -q0, axis=2)       # roll key axis
        if not safe_gate:
            bs = bs * gate[:, None, None]
        m["biasc"] = np.ascontiguousarray(bs.swapaxes(1, 2)).astype(BF_NP)
        if use_mask:
            mr = np.roll(mask[b_idx], -q0)
            kmf = (-10000.0 * (1.0 - mr.astype(np.float32))) + AT_SHIFT
            m["kmc"] = np.ascontiguousarray(
                kmf.reshape(NL, PT).T).astype(np.float32)
            m["mqc"] = mr[:QL].astype(np.float32).reshape(1, QL)
        in_maps.append(m)
    return in_maps, (exp_scales, use_pbias, use_mask)


def kernel(**inputs):
    global LAST_RESULT
    in_maps, (exp_scales, use_pbias, use_mask) = _prep_inputs(**inputs)
    nc = _build_nc(exp_scales, use_pbias, use_mask)
    if not nc.is_finalized():
        nc.finalize()
    res = run_bass_kernel_spmd(nc, in_maps, core_ids=list(range(NCORES)))
    LAST_RESULT = res
    out = np.empty((B, L, E), np.float32)
    for c in range(NCORES):
        b_idx, qh = divmod(c, 2)
        out[b_idx, qh * QL:(qh + 1) * QL, :] = res.results[c]["yc"]
    return out


# revision 25
# speedup vs baseline: 1.0584x; 1.0584x over previous
"""BiasedMultiHeadAttention Trainium2 kernel.

Sharding: 8 cores = (batch b, query-half qh). Each core computes the full
pipeline for its 512 query rows of batch b (K/V projections for the batch
are duplicated across the 2 cores sharing it). No collectives.

Device layout trick: per-core x rows are host-rolled so the core's query
block is always rows 0..511 -> one SPMD program for all 8 cores; bias/mask
are rolled consistently (softmax sum order irrelevant).

Math folding (host, exact):
  xn_aff = ln(x)*g + b folded into weights:  w_eff[i,o] = w[o,i]*ln_g[i]
  b_eff[o] = (w @ ln_b + b)[o];  Q additionally scaled by SCALE/gate_h.

Perf structure (v4):
  - Q/K/V projections in fp8e4 DoubleRow (256-deep contraction = 2x MACs);
    weights host-prescaled by 2^a into e4m3 range, compensated in the
    PSUM->SBUF copy scale.
  - Projection emission is INTERLEAVED with attention head-pairs: after
    K/Q chunks 0-1 and V(half 0) the t=0 attention starts, and each
    remaining K/Q chunk is emitted between head-pairs. The ACT exp train
    (the per-core softmax floor) starts ~40us earlier than with separate
    phases.
  - The bias tile is preloaded into PSUM by an identity matmul
    (start=True), QK accumulates on top, EXP reads PSUM and writes fp8
    at tiles (shifted by AT_SHIFT) that pair two key chunks for
    DoubleRow AV.
  - PSUM plan (8 banks): transposes 1 + proj ring 2 + per-head score
    halves 3 + av pair 2.
"""

import numpy as np
import ml_dtypes

import concourse.bass as bass
import concourse.tile as tile
import concourse.mybir as mybir
from concourse import bacc
from concourse.bass_utils import run_bass_kernel_spmd

B, L, E, H = 4, 1024, 1024, 16
D = E // H
SCALE = D**-0.5
EPS = 1e-5
NCORES = 8
QL = 512  # query rows per core
PT = 128  # partitions
NL = L // PT  # 8 l-chunks
NE = E // PT  # 8 e-chunks
HP = H // 2  # 8 head pairs
NM = NL // 2  # 4 key-chunk pairs

F32 = mybir.dt.float32
BF16 = mybir.dt.bfloat16
F8 = mybir.dt.float8e4
I32 = mybir.dt.int32
BF_NP = ml_dtypes.bfloat16
F8_NP = ml_dtypes.float8_e4m3fn
DR = mybir.MatmulPerfMode.DoubleRow

# host prescale exponents for fp8 weights (compensated in psum->sbuf copy)
QW_S = 32.0   # wq (incl. SCALE/gate fold): sigma ~0.008 -> 0.25
KW_S = 8.0    # wk: sigma ~0.031 -> 0.25
VW_S = 8.0
AT_SHIFT = -3.0   # ACT bias on the exp; keeps fp8 at below e4m3 max

LAST_RESULT = None  # BassKernelResults of the most recent run (for test.py)


def _f8(a):
    return np.clip(np.asarray(a, np.float32), -240.0, 240.0).astype(F8_NP)


def _build_nc(gates, use_pbias, use_mask, dump=()):
    """Build the single-core Bass program (same NEFF for all 8 cores)."""
    nc = bacc.Bacc("TRN2", target_bir_lowering=False, debug=False)
    dump_d = {}

    def dump_tile(name, ap):
        if name in dump:
            d = nc.dram_tensor("d_" + name, list(ap.shape), ap.dtype,
                               kind="ExternalOutput")
            nc.sync.dma_start(d[tuple(slice(None) for _ in ap.shape)], ap)
            dump_d[name] = d

    x_d = nc.dram_tensor("xc", [PT, NL, L], F32, kind="ExternalInput")
    bias_d = nc.dram_tensor("biasc", [H, L, QL], BF16, kind="ExternalInput")
    id_d = nc.dram_tensor("idm", [PT, PT], BF16, kind="ExternalInput")
    wq_d = nc.dram_tensor("wqt", [PT, NE, E], F8, kind="ExternalInput")
    wk_d = nc.dram_tensor("wkt", [PT, NE, E], F8, kind="ExternalInput")
    wv_d = nc.dram_tensor("wvt", [PT, NE, E], F8, kind="ExternalInput")
    wo_d = nc.dram_tensor("wot", [PT, NE, E], BF16, kind="ExternalInput")
    pb_d = {}
    for name, use in zip("qkvo", use_pbias):
        if use:
            pb_d[name] = nc.dram_tensor(f"b{name}e", [1, E], BF16,
                                        kind="ExternalInput")
    if use_mask:
        km_d = nc.dram_tensor("kmc", [PT, NL], F32, kind="ExternalInput")
        mq_d = nc.dram_tensor("mqc", [1, QL], F32, kind="ExternalInput")
    y_d = nc.dram_tensor("yc", [QL, E], F32, kind="ExternalOutput")

    with tile.TileContext(nc) as tc:
        with (
            tc.tile_pool(name="persist", bufs=1) as pp,
            tc.tile_pool(name="consts", bufs=1) as cp,
        ):
            # ---- constants ----
            ident = cp.tile([PT, PT], BF16)
            nc.sync.dma_start(ident, id_d[:, :])
            ones_row = cp.tile([1, L], BF16)
            nc.vector.memset(ones_row, 1.0)
            eps_t = cp.tile([PT, 1], F32)
            nc.vector.memset(eps_t, EPS)
            shift_t = cp.tile([PT, 1], F32)
            nc.vector.memset(shift_t, AT_SHIFT)
            ones65 = cp.tile([65, 64], BF16)
            nc.vector.memset(ones65, 1.0)
            if use_mask:
                km_sb = cp.tile([PT, NL], F32)
                nc.sync.dma_start(km_sb, km_d[:, :])
                mqb = cp.tile([64, QL], F32)
                nc.gpsimd.dma_start(mqb,
                                    mq_d[0:1, :].partition_broadcast(64))

            # ---- resident tensors ----
            x_sb = pp.tile([PT, NL, L], F32)
            for lt in range(NL):
                nc.sync.dma_start(x_sb[:, lt, :], x_d[:, lt, :])
            # weights early so DMA overlaps phase 1
            wq_sb = pp.tile([PT, NE, E], F8)
            nc.sync.dma_start(wq_sb, wq_d[:, :, :])
            wk_sb = pp.tile([PT, NE, E], F8)
            nc.sync.dma_start(wk_sb, wk_d[:, :, :])
            wv_sb = pp.tile([PT, NE, E], F8)
            nc.sync.dma_start(wv_sb, wv_d[:, :, :])
            wo_sb = pp.tile([PT, NE, E], BF16)
            nc.sync.dma_start(wo_sb, wo_d[:, :, :])
            # K^T zero-padded per head parity: full-K=128 QK matmuls with
            # the other head's rows zeroed.
            kTzA = pp.tile([PT, NE, L], BF16)
            kTzB = pp.tile([PT, NE, L], BF16)
            nc.gpsimd.memset(kTzA[64:128, :, :], 0.0)
            nc.gpsimd.memset(kTzB[0:64, :, :], 0.0)
            # V | ones col per head, paired key chunks for DoubleRow AV:
            # v3[:, m, s, h, :] = V rows of key chunk 2m+s
            v3 = pp.tile([PT, NM, 2, H, 65], F8)
            qT = pp.tile([PT, NE, QL], BF16)    # Q^T (scaled) [e_q, q]
            oT = pp.tile([PT, NE, QL], BF16)    # attnout^T (normalized)
            nc.vector.memset(v3[:, :, :, :, 64:65], 1.0)  # ones cols
            pbr = {}
            for name in pb_d:
                pbr[name] = cp.tile([1, E], BF16)
                nc.sync.dma_start(pbr[name], pb_d[name][:, :])

            # ========= Phases 1-3 fused: LN+transpose, projections =======
            # interleaved with attention head-pairs.
            with (
                tc.tile_pool(name="ln", bufs=3) as lp,
                tc.tile_pool(name="xnt", bufs=1) as xp,
                tc.tile_pool(name="pst", bufs=1, space="PSUM") as ptp,
                tc.tile_pool(name="prj", bufs=2, space="PSUM") as prp,
                tc.tile_pool(name="sc", bufs=3, space="PSUM") as scp,
                tc.tile_pool(name="av", bufs=2, space="PSUM") as avp,
                tc.tile_pool(name="bias", bufs=6) as bp,
                tc.tile_pool(name="att2", bufs=4) as a2p,
                tc.tile_pool(name="oo", bufs=6) as oop,
            ):
                xnT = xp.tile([PT, NE, L], F8)  # xn^T [e, l] fp8

                def emit_ln(lt):
                    xr = x_sb[:, lt, :].rearrange("p (s d) -> p s d", s=2)
                    stats = lp.tile([PT, 2, 6], F32, tag="stats")
                    for sg in range(2):
                        nc.vector.bn_stats(stats[:, sg, :], xr[:, sg, :])
                    mv = lp.tile([PT, 2], F32, tag="mv")
                    nc.vector.bn_aggr(mv, stats)
                    sd = lp.tile([PT, 1], F32, tag="sd")
                    nc.scalar.activation(sd, mv[:, 1:2],
                                         mybir.ActivationFunctionType.Sqrt,
                                         bias=eps_t)
                    rs = lp.tile([PT, 1], F32, tag="rs")
                    nc.vector.reciprocal(rs, sd)
                    xnb = lp.tile([PT, L], BF16, tag="xnb")
                    nc.vector.tensor_scalar(
                        out=xnb, in0=x_sb[:, lt, :], scalar1=mv[:, 0:1],
                        scalar2=rs, op0=mybir.AluOpType.subtract,
                        op1=mybir.AluOpType.mult)
                    psT = ptp.tile([PT, L], BF16, tag="psT")
                    for j in range(NE):
                        nc.tensor.transpose(
                            psT[:, j * PT:(j + 1) * PT],
                            xnb[:, j * PT:(j + 1) * PT], ident)
                    nc.scalar.copy(
                        xnT[:, :, lt * PT:(lt + 1) * PT],
                        psT.rearrange("p (j l) -> p j l", j=NE))

                def emit_kq(ot):
                    osl = slice(ot * PT, (ot + 1) * PT)
                    for nh in range(2):  # l halves
                        nsl = slice(nh * 512, (nh + 1) * 512)
                        ps = prp.tile([PT, 512], F32, tag="pp")
                        for j in range(4):
                            nc.tensor.matmul(
                                ps, wk_sb[:, 2 * j:2 * j + 2, osl],
                                xnT[:, 2 * j:2 * j + 2, nsl],
                                start=(j == 0),
                                stop=(j == 3 and "k" not in pbr),
                                perf_mode=DR)
                        if "k" in pbr:
                            nc.tensor.matmul(ps, pbr["k"][:, osl],
                                             ones_row[:, 0:512],
                                             start=False, stop=True)
                        nc.scalar.mul(kTzA[0:64, ot, nsl], ps[0:64, :],
                                      1.0 / KW_S)
                        nc.vector.tensor_scalar_mul(
                            kTzB[64:128, ot, nsl], ps[64:128, :],
                            1.0 / KW_S)
                    # Q (first 512 rolled rows only)
                    psq = prp.tile([PT, 512], F32, tag="pp")
                    for j in range(4):
                        nc.tensor.matmul(
                            psq, wq_sb[:, 2 * j:2 * j + 2, osl],
                            xnT[:, 2 * j:2 * j + 2, 0:512],
                            start=(j == 0),
                            stop=(j == 3 and "q" not in pbr),
                            perf_mode=DR)
                    if "q" in pbr:
                        nc.tensor.matmul(psq, pbr["q"][:, osl],
                                         ones_row[:, 0:512],
                                         start=False, stop=True)
                    nc.scalar.mul(qT[:, ot, 0:512], psq, 1.0 / QW_S)

                def emit_v(lt, vh):
                    lsl = slice(lt * PT, (lt + 1) * PT)
                    vsl = slice(vh * 512, (vh + 1) * 512)
                    psv = prp.tile([PT, 512], F32, tag="pp")
                    for j in range(4):
                        nc.tensor.matmul(
                            psv, xnT[:, 2 * j:2 * j + 2, lsl],
                            wv_sb[:, 2 * j:2 * j + 2, vsl],
                            start=(j == 0),
                            stop=(j == 3 and "v" not in pbr),
                            perf_mode=DR)
                    if "v" in pbr:
                        nc.tensor.matmul(psv, ones_row[:, 0:PT],
                                         pbr["v"][:, vsl],
                                         start=False, stop=True)
                    nc.vector.tensor_scalar_mul(
                        v3[:, lt // 2, lt % 2, vh * 8:(vh + 1) * 8, 0:64],
                        psv.rearrange("p (h d) -> p h d", h=8),
                        1.0 / VW_S)

                def emit_att(t):
                    hA, hB = 2 * t, 2 * t + 1
                    avA = avp.tile([65, QL], F32, tag="av")
                    avB = avp.tile([65, QL], F32, tag="av")
                    at2 = None
                    for c in range(NL):
                        csl = slice(c * PT, (c + 1) * PT)
                        bt = bp.tile([PT, 2, QL], BF16, tag="bt")
                        nc.sync.dma_start(
                            bt, bias_d[hA:hB + 1, csl, :].rearrange(
                                "h p q -> p h q"))
                        if c % 2 == 0:
                            at2 = a2p.tile([PT, 2, 2, QL], F8, tag="at")
                        kmb = (km_sb[:, c:c + 1] if use_mask
                               else shift_t[:, 0:1])
                        for hi, h, ktz in ((0, hA, kTzA), (1, hB, kTzB)):
                            psh = scp.tile([PT, QL], F32, tag="ps")
                            nc.tensor.matmul(psh, ident, bt[:, hi, :],
                                             start=True, stop=False)
                            nc.tensor.matmul(psh, ktz[:, t, csl],
                                             qT[:, t, :],
                                             start=False, stop=True)
                            nc.scalar.activation(
                                at2[:, hi, c % 2, :], psh,
                                mybir.ActivationFunctionType.Exp,
                                bias=kmb, scale=gates[h])
                        if c % 2 == 1:
                            m = c // 2
                            nc.tensor.matmul(
                                avA, v3[:, m, :, hA, :], at2[:, 0, :, :],
                                start=(m == 0), stop=(m == NM - 1),
                                perf_mode=DR)
                            nc.tensor.matmul(
                                avB, v3[:, m, :, hB, :], at2[:, 1, :, :],
                                start=(m == 0), stop=(m == NM - 1),
                                perf_mode=DR)
                    # normalize: SBUF bounce frees the PSUM bank fast; the
                    # rowsum row is broadcast to 64 partitions by a
                    # 1-partition ones matmul (no DRAM roundtrip).
                    for hi, av in enumerate((avA, avB)):
                        avs = oop.tile([65, QL], BF16, tag="avs")
                        nc.vector.tensor_copy(avs, av)
                        rbp = scp.tile([PT, QL], F32, tag="ps")
                        nc.tensor.matmul(rbp[0:64, :], ones65[64:65, :],
                                         avs[64:65, :],
                                         start=True, stop=True)
                        rbs = oop.tile([64, QL], F32, tag="rbs")
                        nc.vector.reciprocal_approx_fast(
                            out=rbs, in_=rbp[0:64, :])
                        if use_mask:
                            nc.vector.tensor_mul(rbs, rbs, mqb)
                        if hi == 0:
                            nc.vector.tensor_mul(oT[0:64, t, :],
                                                 avs[0:64, :], rbs)
                        else:
                            ot_odd = oop.tile([64, QL], BF16, tag="oo")
                            nc.vector.tensor_mul(ot_odd, avs[0:64, :], rbs)
                            nc.sync.dma_start(oT[64:128, t, :], ot_odd)

                # ---- emission order: attention t=0 starts right after
                # K/Q chunks 0-1 and the V half it needs ----
                for lt in range(NL):
                    emit_ln(lt)
                emit_kq(0)
                emit_kq(1)
                for lt in range(NL):
                    emit_v(lt, 0)
                emit_att(0)
                emit_kq(2)
                for lt in range(NL):
                    emit_v(lt, 1)
                emit_att(1)
                for t in range(2, HP):
                    if t + 1 < HP:
                        emit_kq(t + 1)
                    emit_att(t)

            dump_tile("oT", oT[:, :, :])

            # ====== Phase 4: out-proj directly in [q, e] + residual ======
            with (
                tc.tile_pool(name="fp", bufs=2, space="PSUM") as fpp,
                tc.tile_pool(name="yo", bufs=2) as yop,
            ):
                for qb in range(4):
                    qsl = slice(qb * PT, (qb + 1) * PT)
                    psf = fpp.tile([PT, E], F32, tag="pf")
                    for eh in range(2):
                        esl = slice(eh * 512, (eh + 1) * 512)
                        for ic in range(NE):
                            nc.tensor.matmul(
                                psf[:, esl], oT[:, ic, qsl],
                                wo_sb[:, ic, esl], start=(ic == 0),
                                stop=(ic == NE - 1 and "o" not in pbr))
                        if "o" in pbr:
                            nc.tensor.matmul(psf[:, esl],
                                             ones_row[0:1, 0:PT],
                                             pbr["o"][:, esl],
                                             start=False, stop=True)
                    y_sb = yop.tile([PT, E], F32, tag="y")
                    nc.vector.tensor_add(y_sb, psf, x_sb[:, qb, :])
                    nc.sync.dma_start(y_d[qsl, :], y_sb)
    return nc


def _prep_inputs(x, bias, mask, wq, bq, wk, bk, wv, bv, wo, bo, gate,
                 ln_g, ln_b):
    """Host-side folding + per-core sharding. Returns (in_maps, meta)."""
    gate = np.asarray(gate, np.float32)
    ln_g = np.asarray(ln_g, np.float32)
    ln_b = np.asarray(ln_b, np.float32)
    grep = np.repeat(gate, D)  # [E]
    safe_gate = bool(np.all(np.abs(gate) > 1e-6))
    if safe_gate:
        qscale = (SCALE / grep).astype(np.float32)
        exp_scales = [float(g) for g in gate]
    else:
        # fold gate into bias on host instead (gate ~ 0 edge case)
        qscale = np.full(E, SCALE, np.float32)
        exp_scales = [1.0] * H

    wqt = _f8(np.asarray(wq).T * ln_g[:, None] * qscale[None, :] * QW_S)
    wkt = _f8(np.asarray(wk).T * ln_g[:, None] * KW_S)
    wvt = _f8(np.asarray(wv).T * ln_g[:, None] * VW_S)
    wot = np.asarray(wo).T.astype(BF_NP)
    bqe = ((np.asarray(wq) @ ln_b + np.asarray(bq)) * qscale * QW_S
           ).astype(np.float32)
    bke = ((np.asarray(wk) @ ln_b + np.asarray(bk)) * KW_S).astype(np.float32)
    bve = ((np.asarray(wv) @ ln_b + np.asarray(bv)) * VW_S).astype(np.float32)
    boe = np.asarray(bo, np.float32)
    use_pbias = tuple(bool(np.any(b)) for b in (bqe, bke, bve, boe))

    mask = np.asarray(mask, np.int32)
    use_mask = not bool(np.all(mask == 1))

    def wfmt(w):  # [E_in, E_out] -> [128, 8, E]
        return np.ascontiguousarray(
            w.reshape(NE, PT, E).transpose(1, 0, 2))

    shared = {"wqt": wfmt(wqt), "wkt": wfmt(wkt), "wvt": wfmt(wvt),
              "wot": wfmt(wot),
              "idm": np.eye(PT, dtype=BF_NP)}
    for name, use, b in zip("qkvo", use_pbias, (bqe, bke, bve, boe)):
        if use:
            shared[f"b{name}e"] = b.reshape(1, E).astype(BF_NP)

    x = np.asarray(x, np.float32)
    bias = np.asarray(bias, np.float32)
    in_maps = []
    for c in range(NCORES):
        b_idx, qh = divmod(c, 2)
        q0 = qh * QL
        xr = np.roll(x[b_idx], -q0, axis=0)  # query block first
        m = {}
        m.update(shared)
        m["xc"] = np.ascontiguousarray(
            xr.reshape(NL, PT, L).transpose(1, 0, 2))
        bs = bias[b_idx][:, q0:q0 + QL, :]  # [H, QL, L]
        bs = np.roll(bs, -q0, axis=2)       # roll key axis
        if not safe_gate:
            bs = bs * gate[:, None, None]
        m["biasc"] = np.ascontiguousarray(bs.swapaxes(1, 2)).astype(BF_NP)
        if use_mask:
            mr = np.roll(mask[b_idx], -q0)
            kmf = (-10000.0 * (1.0 - mr.astype(np.float32))) + AT_SHIFT
            m["kmc"] = np.ascontiguousarray(
                kmf.reshape(NL, PT).T).astype(np.float32)
            m["mqc"] = mr[:QL].astype(np.float32).reshape(1, QL)
        in_maps.append(m)
    return in_maps, (exp_scales, use_pbias, use_mask)


def kernel(**inputs):
    global LAST_RESULT
    in_maps, (exp_scales, use_pbias, use_mask) = _prep_inputs(**inputs)
    nc = _build_nc(exp_scales, use_pbias, use_mask)
    if not nc.is_finalized():
        nc.finalize()
    res = run_bass_kernel_spmd(nc, in_maps, core_ids=list(range(NCORES)))
    LAST_RESULT = res
    out = np.empty((B, L, E), np.float32)
    for c in range(NCORES):
        b_idx, qh = divmod(c, 2)
        out[b_idx, qh * QL:(qh + 1) * QL, :] = res.results[c]["yc"]
    return out
